# revision 13
# baseline (speedup 1.0000x reference)
"""Trainium2 Bass kernel for nn_Conv_6511170421767.

3x3 conv, stride 1, pad 1 on x:(32,128,56,56) with weight:(256,128,3,3),
bias:(256,) -> out:(32,256,56,56), fp32 in/out.

Strategy (data-parallel, 4 images per core on 8 cores):
- Cin=128 is exactly the PE contraction/partition dim. The conv becomes,
  per (output-row-block, Cout-chunk), an accumulation of 9 matmuls (one per
  kernel tap): out[co, pix] += W[dr,dc][ci,co].T @ xpad[ci, shifted pix].
- The host pre-pads x to (4,128,58,58) fp16 and pre-transposes/casts the
  weights, so input DMAs land directly in the padded SBUF plane and the
  on-chip DVE does ZERO work (v1 spent ~26us of DVE on casts/borders).
- Matmul operands are fp16 (1 PE cycle/row - vs 4 for plain fp32 - with a
  10-bit mantissa; operand ranges here sit safely inside fp16's range).
  Accumulation is fp32 in PSUM. Measured rel err vs fp32 reference: ~3e-4.
- PSUM tile [128, 448] = one bank; 9 taps accumulate in-bank, then the
  scalar engine adds bias (Identity activation w/ per-partition bias AP)
  while copying PSUM->SBUF as fp16; result DMAs out on one of three
  round-robin queues (sync/vector/gpsimd) so no single DMA queue backs up
  (v1 put all 56 output DMAs on sync: 92 pending descriptors at the end,
  ~5us drain tail). The host upcasts fp16->fp32 (+bias is folded on-chip).
- Cold-start: the PE HAM clock gate runs matmuls at 1.2GHz until ~3.4us of
  sustained PE-busy. Dependency-free warmup matmuls (zero fp16 operands
  from a DVE memset, never read back) start right after the ~7.5us fixed
  framework preamble, so the HAM flips to 2.4GHz earlier into the real
  stream. img0's input DMA is split into 4 row-band chunks so the first
  real matmul only waits on an 18-row chunk.

The external neuronxcc walrus in this container enforces small per-
instruction sync-wait limits (TRN2 HW allows 1 per instruction);
_cap_sync_waits() splits excess waits onto InstNoOp instructions inserted
just before the offender on the same engine.
"""

import sys

sys.path.insert(0, "/opt/trn_rl_repo")

import numpy as np

import concourse.bass as bass
import concourse.mybir as mybir
import concourse.tile as tile
from concourse.bass_utils import run_bass_kernel_spmd

F32 = mybir.dt.float32
FP16 = mybir.dt.float16

N_CORES = 8
IMGS_PER_CORE = 4
CIN = 128
COUT = 256
H = W = 56
HP = WP = 58  # padded plane
ROWS_PER_TILE = 8  # 8 output rows -> N = 448 <= 512 (one PSUM bank)
N_ROW_TILES = H // ROWS_PER_TILE  # 7
NTILE = ROWS_PER_TILE * W  # 448
N_WARM = 6  # dependency-free HAM-warmup matmuls
IMG0_CHUNKS = [0, 10, 26, 42, 58]  # row bands; tile t needs rows 8t..8t+10

# Per-instruction sync-wait budget for the external walrus: TRN2 hardware
# allows at most 1 sync wait per instruction.
_WAIT_LIMITS_DEFAULT = 1
_WAIT_LIMITS = {}


def _cap_sync_waits(nc):
    """Split sync waits exceeding per-instruction limits onto same-engine
    InstNoOp instructions inserted immediately before the offender."""
    for fn in nc.m.functions:
        for bb in fn.blocks:
            i = 0
            insts = bb.instructions
            while i < len(insts):
                inst = insts[i]
                si = getattr(inst, "sync_info", None)
                if si is None or not si.on_wait:
                    i += 1
                    continue
                limit = _WAIT_LIMITS.get(type(inst).__name__, _WAIT_LIMITS_DEFAULT)
                waits = list(si.on_wait)
                if len(waits) <= limit:
                    i += 1
                    continue
                keep = waits[:limit]
                excess = waits[limit:]
                inst.sync_info = mybir.SyncInfo(
                    on_wait=keep, on_update=list(si.on_update)
                )
                pos = i
                for j in range(0, len(excess), _WAIT_LIMITS_DEFAULT):
                    chunk = excess[j : j + _WAIT_LIMITS_DEFAULT]
                    nop = mybir.InstNoOp(
                        name=nc.get_next_instruction_name(), ins=[], outs=[]
                    )
                    nop.engine = inst.engine
                    nop.sync_info = mybir.SyncInfo(on_wait=chunk, on_update=[])
                    nc.register_instruction(nop)
                    insts.insert(pos, nop)
                    pos += 1
                    i += 1
                i += 1


def build_conv_nc():
    """One-core program: x:(4,128,58,58) fp16 (pre-padded), wT:(128,9*256)
    fp16, bias2:(128,2) f32 -> out:(4,256,56,56) fp16."""
    nc = bass.Bass()
    x = nc.dram_tensor("x", [IMGS_PER_CORE, CIN, HP, WP], FP16, kind="ExternalInput")
    wt = nc.dram_tensor("wT", [CIN, 9 * COUT], FP16, kind="ExternalInput")
    bias2 = nc.dram_tensor("bias2", [128, 2], F32, kind="ExternalInput")
    out = nc.dram_tensor(
        "out", [IMGS_PER_CORE, COUT, H, W], FP16, kind="ExternalOutput"
    )

    with tile.TileContext(nc) as tc:
        with (
            tc.tile_pool(name="const", bufs=1) as const_pool,
            tc.tile_pool(name="xpad", bufs=1) as xpad_pool,
            tc.tile_pool(name="obuf", bufs=4) as obuf_pool,
            tc.tile_pool(name="psum", bufs=8, space="PSUM") as psum_pool,
        ):
            w_sb = const_pool.tile([CIN, 9 * COUT], FP16)
            b_sb = const_pool.tile([128, 2], F32)
            wz = const_pool.tile([CIN, NTILE], FP16)
            xpads = [
                xpad_pool.tile([CIN, HP, WP], FP16, tag=f"xpad{i}", name=f"xpad{i}")
                for i in range(3)
            ]

            # HAM warmup: memset-only dependency, so these issue right after
            # the framework preamble and keep the PE busy while input DMAs
            # stream. Results are never read.
            nc.vector.memset(wz[:], 0.0)
            for i in range(N_WARM):
                pw = psum_pool.tile([128, NTILE], F32, tag="ps", name=f"warm{i}")
                nc.tensor.matmul(pw[:], wz[:, 0:128], wz[:], start=True, stop=True)

            # Startup DMAs run on BOTH HWDGE rings in parallel, ordered by
            # first-use deadline (tap k of tile0 is needed ~0.4us*k after
            # MM#1; each transfer's completion semaphore lags its data by
            # ~1us, so weights move in three deadline-sized pieces).
            # Layout wT[ci, (tap, chunk, co128)].
            nc.sync.dma_start(xpads[0][:, 0:10, :], x[0, :, 0:10, :])
            nc.sync.dma_start(w_sb[:, 10 * 128 :], wt[:, 10 * 128 :])
            nc.sync.dma_start(
                xpads[0][:, 10 : IMG0_CHUNKS[2], :], x[0, :, 10 : IMG0_CHUNKS[2], :]
            )
            nc.sync.dma_start(b_sb[:], bias2[:])
            nc.scalar.dma_start(w_sb[:, 0 : 4 * 128], wt[:, 0 : 4 * 128])
            nc.scalar.dma_start(w_sb[:, 4 * 128 : 10 * 128], wt[:, 4 * 128 : 10 * 128])
            for ci in range(2, len(IMG0_CHUNKS) - 1):
                r0, r1 = IMG0_CHUNKS[ci], IMG0_CHUNKS[ci + 1]
                nc.sync.dma_start(xpads[0][:, r0:r1, :], x[0, :, r0:r1, :])
            # img1/2 prefetch rides the sync ring: on the scalar ring their
            # bulk delayed the taps-2-4 weight completion past its deadline
            # (k2 stalled 3.2us and HAM re-throttled).
            nc.sync.dma_start(xpads[1][:], x[1])
            nc.sync.dma_start(xpads[2][:], x[2])

            for img in range(IMGS_PER_CORE):
                xp = xpads[img % 3]
                for t in range(N_ROW_TILES):
                    if img == 1 and t == 2:
                        # img3 -> xpads[0]; by the time the scalar queue
                        # reaches this, img0 is fully consumed (no stall).
                        nc.scalar.dma_start(xpads[0][:], x[3])
                    y0 = t * ROWS_PER_TILE
                    # The very last tile is split into row-halves so the
                    # final ACT+DMA+completion chain after the last matmul
                    # is half as deep.
                    last = img == IMGS_PER_CORE - 1 and t == N_ROW_TILES - 1
                    spans = (
                        [(y0, ROWS_PER_TILE // 2), (y0 + 4, ROWS_PER_TILE // 2)]
                        if last
                        else [(y0, ROWS_PER_TILE)]
                    )
                    for c in range(2):  # Cout chunks of 128
                        for r0, nr in spans:
                            nt = nr * W
                            ps = psum_pool.tile(
                                [128, nt], F32, tag="ps", name=f"ps_{img}_{r0}_{c}"
                            )
                            for k in range(9):
                                dr, dc = divmod(k, 3)
                                lhsT = w_sb[
                                    :, (k * 2 + c) * 128 : (k * 2 + c) * 128 + 128
                                ]
                                rhs = xp[:, r0 + dr : r0 + dr + nr, dc : dc + W]
                                nc.tensor.matmul(
                                    ps[:], lhsT, rhs, start=(k == 0), stop=(k == 8)
                                )
                            ob = obuf_pool.tile(
                                [128, nr, W], FP16, tag="ob",
                                name=f"ob_{img}_{r0}_{c}",
                            )
                            # out = Identity(psum * 1.0 + bias[co]) on ScalarE
                            nc.scalar.activation(
                                ob[:],
                                ps[:].rearrange("p (r w) -> p r w", w=W),
                                mybir.ActivationFunctionType.Identity,
                                bias=b_sb[:, c : c + 1],
                                scale=1.0,
                            )
                            nc.sync.dma_start(
                                out[img, c * 128 : (c + 1) * 128, r0 : r0 + nr, :],
                                ob[:],
                            )

    _cap_sync_waits(nc)
    nc.finalize()
    return nc


_NC_CACHE = {}


def _get_nc():
    if "nc" not in _NC_CACHE:
        _NC_CACHE["nc"] = build_conv_nc()
    return _NC_CACHE["nc"]


def _prep_in_maps(x, weight, bias):
    x = np.asarray(x, dtype=np.float32)
    n = x.shape[0]
    # pad to 58x58 and cast fp16 once, full batch
    xp = np.zeros((n, CIN, HP, WP), dtype=np.float16)
    xp[:, :, 1 : H + 1, 1 : W + 1] = x
    # weight (256,128,3,3) -> wT[ci, (tap, chunk, co128)] fp16
    wT = (
        np.transpose(np.asarray(weight, dtype=np.float32), (1, 2, 3, 0))
        .reshape(CIN, 9, 2, 128)
        .reshape(CIN, 9 * COUT)
        .astype(np.float16)
    )
    wT = np.ascontiguousarray(wT)
    bias2 = np.ascontiguousarray(
        np.asarray(bias, dtype=np.float32).reshape(2, 128).T
    )
    per_core = n // N_CORES
    return [
        {
            "x": np.ascontiguousarray(xp[i * per_core : (i + 1) * per_core]),
            "wT": wT,
            "bias2": bias2,
        }
        for i in range(N_CORES)
    ]


def run(x, weight, bias, trace=False):
    """Run the conv on 8 cores; returns (out, BassKernelResults)."""
    nc = _get_nc()
    in_maps = _prep_in_maps(x, weight, bias)
    res = run_bass_kernel_spmd(
        nc, in_maps, core_ids=list(range(N_CORES)), trace=trace
    )
    out = np.concatenate([r["out"] for r in res.results], axis=0).astype(np.float32)
    return out, res


def kernel(x, weight, bias):
    out, _ = run(x, weight, bias, trace=False)
    return out


# revision 14
# speedup vs baseline: 1.1567x; 1.1567x over previous
"""Trainium2 Bass kernel for nn_Conv_6511170421767.

3x3 conv, stride 1, pad 1 on x:(32,128,56,56) with weight:(256,128,3,3),
bias:(256,) -> out:(32,256,56,56), fp32 in/out.

Strategy (data-parallel, 4 images per core on 8 cores):
- Cin=128 is exactly the PE contraction/partition dim. The conv becomes,
  per (output-row-block, Cout-chunk), an accumulation of 9 matmuls (one per
  kernel tap): out[co, pix] += W[dr,dc][ci,co].T @ xpad[ci, shifted pix].
- The host pre-pads x to (4,128,58,58) fp16 and pre-transposes/casts the
  weights, so input DMAs land directly in the padded SBUF plane and the
  on-chip DVE does ZERO work (v1 spent ~26us of DVE on casts/borders).
- Matmul operands are fp16 (1 PE cycle/row - vs 4 for plain fp32 - with a
  10-bit mantissa; operand ranges here sit safely inside fp16's range).
  Accumulation is fp32 in PSUM. Measured rel err vs fp32 reference: ~3e-4.
- PSUM tile [128, 448] = one bank; 9 taps accumulate in-bank, then the
  scalar engine adds bias (Identity activation w/ per-partition bias AP)
  while copying PSUM->SBUF as fp16; the host upcasts fp16->fp32. The very
  last tile is split into row-halves to shorten the final ACT+DMA+
  completion chain. Output DMAs ride the sync ring (it keeps pace; ~0.6us
  descriptor cost each, paced ~1.7us apart by compute).
- Cold-start: the PE HAM clock gate runs matmuls at 1.2GHz until ~3.4us of
  CONTINUOUS PE-busy (any idle gap restarts the window). Dependency-free
  warmup matmuls (zero fp16 operands from a DVE memset, never read back)
  start right after the ~7.5us fixed framework preamble and bridge into
  the first data-gated matmul, so the HAM flips to 2.4GHz ~1.8us into the
  real stream. Startup transfers are deadline-ordered across BOTH HWDGE
  rings: sync = [x rows 0-9, taps 5-8, x rows 10-25, bias, x rest, img1,
  img2], scalar = [taps 0-1, taps 2-4] (kept free of bulk so the tap
  completion semaphores land before their first LDWEIGHTS; a transfer's
  completion lags its issue by ~2-2.5us). Each DMA's completion semaphore
  is per-transfer, hence the deadline-sized pieces.

The external neuronxcc walrus in this container enforces small per-
instruction sync-wait limits (TRN2 HW allows 1 per instruction);
_cap_sync_waits() splits excess waits onto InstNoOp instructions inserted
just before the offender on the same engine.
"""

import sys

sys.path.insert(0, "/opt/trn_rl_repo")

import numpy as np

import concourse.bass as bass
import concourse.mybir as mybir
import concourse.tile as tile
from concourse.bass_utils import run_bass_kernel_spmd

F32 = mybir.dt.float32
FP16 = mybir.dt.float16

N_CORES = 8
IMGS_PER_CORE = 4
CIN = 128
COUT = 256
H = W = 56
HP = WP = 58  # padded plane
ROWS_PER_TILE = 8  # 8 output rows -> N = 448 <= 512 (one PSUM bank)
N_ROW_TILES = H // ROWS_PER_TILE  # 7
NTILE = ROWS_PER_TILE * W  # 448
N_WARM = 6  # dependency-free HAM-warmup matmuls
IMG0_CHUNKS = [0, 10, 26, 42, 58]  # row bands; tile t needs rows 8t..8t+10

# Per-instruction sync-wait budget for the external walrus: TRN2 hardware
# allows at most 1 sync wait per instruction.
_WAIT_LIMITS_DEFAULT = 1
_WAIT_LIMITS = {}


def _cap_sync_waits(nc):
    """Split sync waits exceeding per-instruction limits onto same-engine
    InstNoOp instructions inserted immediately before the offender."""
    for fn in nc.m.functions:
        for bb in fn.blocks:
            i = 0
            insts = bb.instructions
            while i < len(insts):
                inst = insts[i]
                si = getattr(inst, "sync_info", None)
                if si is None or not si.on_wait:
                    i += 1
                    continue
                limit = _WAIT_LIMITS.get(type(inst).__name__, _WAIT_LIMITS_DEFAULT)
                waits = list(si.on_wait)
                if len(waits) <= limit:
                    i += 1
                    continue
                keep = waits[:limit]
                excess = waits[limit:]
                inst.sync_info = mybir.SyncInfo(
                    on_wait=keep, on_update=list(si.on_update)
                )
                pos = i
                for j in range(0, len(excess), _WAIT_LIMITS_DEFAULT):
                    chunk = excess[j : j + _WAIT_LIMITS_DEFAULT]
                    nop = mybir.InstNoOp(
                        name=nc.get_next_instruction_name(), ins=[], outs=[]
                    )
                    nop.engine = inst.engine
                    nop.sync_info = mybir.SyncInfo(on_wait=chunk, on_update=[])
                    nc.register_instruction(nop)
                    insts.insert(pos, nop)
                    pos += 1
                    i += 1
                i += 1


def build_conv_nc():
    """One-core program: x:(4,128,58,58) fp16 (pre-padded), wT:(128,9*256)
    fp16, bias2:(128,2) f32 -> out:(4,256,56,56) fp16."""
    nc = bass.Bass()
    x = nc.dram_tensor("x", [IMGS_PER_CORE, CIN, HP, WP], FP16, kind="ExternalInput")
    wt = nc.dram_tensor("wT", [CIN, 9 * COUT], FP16, kind="ExternalInput")
    bias2 = nc.dram_tensor("bias2", [128, 2], F32, kind="ExternalInput")
    out = nc.dram_tensor(
        "out", [IMGS_PER_CORE, COUT, H, W], FP16, kind="ExternalOutput"
    )

    with tile.TileContext(nc) as tc:
        with (
            tc.tile_pool(name="const", bufs=1) as const_pool,
            tc.tile_pool(name="xpad", bufs=1) as xpad_pool,
            tc.tile_pool(name="obuf", bufs=4) as obuf_pool,
            tc.tile_pool(name="psum", bufs=8, space="PSUM") as psum_pool,
        ):
            w_sb = const_pool.tile([CIN, 9 * COUT], FP16)
            b_sb = const_pool.tile([128, 2], F32)
            wz = const_pool.tile([CIN, NTILE], FP16)
            xpads = [
                xpad_pool.tile([CIN, HP, WP], FP16, tag=f"xpad{i}", name=f"xpad{i}")
                for i in range(3)
            ]

            # HAM warmup: memset-only dependency, so these issue right after
            # the framework preamble and keep the PE busy while input DMAs
            # stream. Results are never read.
            nc.vector.memset(wz[:], 0.0)
            for i in range(N_WARM):
                pw = psum_pool.tile([128, NTILE], F32, tag="ps", name=f"warm{i}")
                nc.tensor.matmul(pw[:], wz[:, 0:128], wz[:], start=True, stop=True)

            # Startup DMAs run on BOTH HWDGE rings in parallel, ordered by
            # first-use deadline (tap k of tile0 is needed ~0.4us*k after
            # MM#1; each transfer's completion semaphore lags its data by
            # ~1us, so weights move in three deadline-sized pieces).
            # Layout wT[ci, (tap, chunk, co128)].
            nc.sync.dma_start(xpads[0][:, 0:10, :], x[0, :, 0:10, :])
            nc.sync.dma_start(w_sb[:, 10 * 128 :], wt[:, 10 * 128 :])
            nc.sync.dma_start(
                xpads[0][:, 10 : IMG0_CHUNKS[2], :], x[0, :, 10 : IMG0_CHUNKS[2], :]
            )
            nc.sync.dma_start(b_sb[:], bias2[:])
            nc.scalar.dma_start(w_sb[:, 0 : 4 * 128], wt[:, 0 : 4 * 128])
            nc.scalar.dma_start(w_sb[:, 4 * 128 : 10 * 128], wt[:, 4 * 128 : 10 * 128])
            for ci in range(2, len(IMG0_CHUNKS) - 1):
                r0, r1 = IMG0_CHUNKS[ci], IMG0_CHUNKS[ci + 1]
                nc.sync.dma_start(xpads[0][:, r0:r1, :], x[0, :, r0:r1, :])
            # img1/2 prefetch rides the sync ring: on the scalar ring their
            # bulk delayed the taps-2-4 weight completion past its deadline
            # (k2 stalled 3.2us and HAM re-throttled).
            nc.sync.dma_start(xpads[1][:], x[1])
            nc.sync.dma_start(xpads[2][:], x[2])

            for img in range(IMGS_PER_CORE):
                xp = xpads[img % 3]
                for t in range(N_ROW_TILES):
                    if img == 1 and t == 2:
                        # img3 -> xpads[0]; by the time the scalar queue
                        # reaches this, img0 is fully consumed (no stall).
                        nc.scalar.dma_start(xpads[0][:], x[3])
                    y0 = t * ROWS_PER_TILE
                    # The very last tile is split into row-halves so the
                    # final ACT+DMA+completion chain after the last matmul
                    # is half as deep.
                    last = img == IMGS_PER_CORE - 1 and t == N_ROW_TILES - 1
                    spans = (
                        [(y0, ROWS_PER_TILE // 2), (y0 + 4, ROWS_PER_TILE // 2)]
                        if last
                        else [(y0, ROWS_PER_TILE)]
                    )
                    for c in range(2):  # Cout chunks of 128
                        for r0, nr in spans:
                            nt = nr * W
                            ps = psum_pool.tile(
                                [128, nt], F32, tag="ps", name=f"ps_{img}_{r0}_{c}"
                            )
                            for k in range(9):
                                dr, dc = divmod(k, 3)
                                lhsT = w_sb[
                                    :, (k * 2 + c) * 128 : (k * 2 + c) * 128 + 128
                                ]
                                rhs = xp[:, r0 + dr : r0 + dr + nr, dc : dc + W]
                                nc.tensor.matmul(
                                    ps[:], lhsT, rhs, start=(k == 0), stop=(k == 8)
                                )
                            ob = obuf_pool.tile(
                                [128, nr, W], FP16, tag="ob",
                                name=f"ob_{img}_{r0}_{c}",
                            )
                            # out = Identity(psum * 1.0 + bias[co]) on ScalarE
                            nc.scalar.activation(
                                ob[:],
                                ps[:].rearrange("p (r w) -> p r w", w=W),
                                mybir.ActivationFunctionType.Identity,
                                bias=b_sb[:, c : c + 1],
                                scale=1.0,
                            )
                            nc.sync.dma_start(
                                out[img, c * 128 : (c + 1) * 128, r0 : r0 + nr, :],
                                ob[:],
                            )

    _cap_sync_waits(nc)
    nc.finalize()
    return nc


_NC_CACHE = {}


def _get_nc():
    if "nc" not in _NC_CACHE:
        _NC_CACHE["nc"] = build_conv_nc()
    return _NC_CACHE["nc"]


def _prep_in_maps(x, weight, bias):
    x = np.asarray(x, dtype=np.float32)
    n = x.shape[0]
    # pad to 58x58 and cast fp16 once, full batch
    xp = np.zeros((n, CIN, HP, WP), dtype=np.float16)
    xp[:, :, 1 : H + 1, 1 : W + 1] = x
    # weight (256,128,3,3) -> wT[ci, (tap, chunk, co128)] fp16
    wT = (
        np.transpose(np.asarray(weight, dtype=np.float32), (1, 2, 3, 0))
        .reshape(CIN, 9, 2, 128)
        .reshape(CIN, 9 * COUT)
        .astype(np.float16)
    )
    wT = np.ascontiguousarray(wT)
    bias2 = np.ascontiguousarray(
        np.asarray(bias, dtype=np.float32).reshape(2, 128).T
    )
    per_core = n // N_CORES
    return [
        {
            "x": np.ascontiguousarray(xp[i * per_core : (i + 1) * per_core]),
            "wT": wT,
            "bias2": bias2,
        }
        for i in range(N_CORES)
    ]


def run(x, weight, bias, trace=False):
    """Run the conv on 8 cores; returns (out, BassKernelResults)."""
    nc = _get_nc()
    in_maps = _prep_in_maps(x, weight, bias)
    res = run_bass_kernel_spmd(
        nc, in_maps, core_ids=list(range(N_CORES)), trace=trace
    )
    out = np.concatenate([r["out"] for r in res.results], axis=0).astype(np.float32)
    return out, res


def kernel(x, weight, bias):
    out, _ = run(x, weight, bias, trace=False)
    return out
